# revision 10
# baseline (speedup 1.0000x reference)
"""Trainium2 Bass kernel for nn_BoundaryDiceLoss_82171314307268.

Strategy (data-parallel over 8 cores): sample s = c//2, D-half = c%2.
Each core receives a [H=128(part), 70 D-slots, 128 w] slab (owned 64 D
slices + 3-deep halo, out-of-volume replicated with edge values) of
out0 = output[s,0], out1 = output[s,1], tgt = target[s,0].

On-core algorithm (all mask arithmetic exact small-integer bf16):
  diff  = out1 - out0 ;  probs = sigmoid(diff) (owned slices)
  mc    = (diff > 0) + 2*tgt            in {0,1,2,3}; combined 4-state field
  boundary-ness E[v] = sum_{6-nbrs} (mc[v]-mc[nbr])^2
                     = mc*(6*mc - 2*c) + q,   c = S6(mc), q = S6(mc^2)
    (edge-replicated padding makes every voxel have 6 neighbors: host pads
     the D axis, column copies pad W, and the A1 band matrix replicates H
     edges, so out-of-volume "neighbors" contribute zero difference --
     exactly the in-volume-only neighbor-diff semantics of the reference.)
  E >= 1 iff boundary voxel of gt-mask OR pred-mask (else 0).
  region = conv3d(E, ball radius 2) > 0.5, ball decomposed into 7 PE terms:
     T5@E + T3@s3z + T3@F[w-1] + T3@F[w+1] + I@c4b + I@E[w-2] + I@E[w+2]
     with s3z = E[z-1]+E[z+1], F = E + s3z, c4b = E[z-2]+E[z+2]
  partial sums over owned region (per-partition, fused accumulate):
     S_pm = sum probs*m, S_ptm2 = sum probs*(2 tgt)*m, S_tm2 = sum 2 tgt*m,
     S_m = sum m           -> [128,4] f32 -> host combines + dice math.

H-axis (partition) neighbor sums go through the tensor engine as banded
128x128 matmuls; w/z-axis shifts are free-dim AP offsets on DVE/GPSIMD.
"""
import sys

sys.path.insert(0, "/opt/trn_rl_repo")

import numpy as np
import ml_dtypes

import concourse.bass as bass
import concourse.bacc as bacc
import concourse.tile as tile
import concourse.mybir as mybir
from concourse.bass_utils import run_bass_kernel_spmd

f32 = mybir.dt.float32
bf16 = mybir.dt.bfloat16
Alu = mybir.AluOpType
Act = mybir.ActivationFunctionType

P = 128          # H on partitions
D_VOL = 128      # volume depth
W = 128
OWN = 64         # owned D slices per core
HALO = 3
DEXT = OWN + 2 * HALO          # 70 slab D-slots
WP = W + 4                     # padded w stride, data cols [2, 130)
B = 4
EPS = 1e-05

CH = 4                         # D-slots per chunk (512 free elems)
N_A = (DEXT + CH - 1) // CH    # 18 phase-A chunks (last has 2 slots)
BLO, BHI = 1, 69               # E/t1 computed on slots [1,69)
OLO, OHI = 3, 67               # owned slots


def _band(offsets, rep_edges=False):
    m = np.zeros((P, P), np.float32)
    for o in offsets:
        for i in range(P):
            j = i + o
            if 0 <= j < P:
                m[j, i] += 1.0
            elif rep_edges:
                m[min(max(j, 0), P - 1), i] += 1.0
    return m


def _const_mats():
    a1 = _band([-1, 1], rep_edges=True)       # H-neighbor sum, edges replicated
    t3 = _band([-1, 0, 1])
    t5 = _band([-2, -1, 0, 1, 2])
    ident = np.eye(P, dtype=np.float32)
    m_mc = 6.0 * ident - 2.0 * a1
    m_n2i = -2.0 * ident
    return {"m_mc": m_mc, "m_a1": a1, "m_t3": t3, "m_t5": t5,
            "m_id": ident, "m_n2i": m_n2i}


def _build_program():
    nc = bacc.Bacc("TRN2", target_bir_lowering=False, debug=False,
                   num_devices=8)
    d_out0 = nc.dram_tensor("out0", [P, DEXT * W], f32, kind="ExternalInput")
    d_out1 = nc.dram_tensor("out1", [P, DEXT * W], f32, kind="ExternalInput")
    d_tgt = nc.dram_tensor("tgt", [P, DEXT * W], f32, kind="ExternalInput")
    d_mats = {n: nc.dram_tensor(n, [P, P], bf16, kind="ExternalInput")
              for n in ("m_mc", "m_a1", "m_t3", "m_t5", "m_id", "m_n2i")}
    d_psums = nc.dram_tensor("psums", [P, 4], f32, kind="ExternalOutput")

    with tile.TileContext(nc) as tc:
        with tc.tile_pool(name="consts", bufs=1) as cp, \
             tc.tile_pool(name="slabs", bufs=1) as sp, \
             tc.tile_pool(name="chunks", bufs=3) as kp, \
             tc.tile_pool(name="ps_t1", bufs=2, space="PSUM") as ps_t1, \
             tc.tile_pool(name="ps_e", bufs=2, space="PSUM") as ps_e, \
             tc.tile_pool(name="ps_p", bufs=2, space="PSUM") as ps_p:

            mats = {}
            for n in d_mats:
                mats[n] = cp.tile([P, P], bf16, tag=n, name=n)
                nc.sync.dma_start(mats[n][:], d_mats[n][:])

            def slab(name_, cols=WP, dtype=bf16, slots=DEXT, tag_override=None):
                t = sp.tile([P, slots * cols], dtype,
                            tag=tag_override or name_, name=name_)
                return t.rearrange("p (s w) -> p s w", w=cols)

            tgt2 = slab("tgt2")              # 2*tgt, padded layout
            mc = slab("mc")
            sq = slab("sq")
            tb = slab("tb")                  # mc[z-1]+mc[z+1]
            tbq = slab("tbq")                # sq[z-1]+sq[z+1]
            t1 = slab("t1", cols=W, slots=68)    # 6mc-2c, packed, slot i -> 1+i
            etmp = slab("etmp", cols=W, slots=68)
            e3 = slab("e", cols=WP)
            probs = slab("probs", cols=W, slots=OWN)   # slot i -> 3+i

            # ---- phase A: stream chunks, build diff/mc/tgt2/probs ----
            for k in range(N_A):
                s0 = k * CH
                ns = min(CH, DEXT - s0)
                sl = slice(s0, s0 + ns)
                c0 = kp.tile([P, CH * W], f32, tag="out0c")
                c1 = kp.tile([P, CH * W], f32, tag="out1c")
                ct = kp.tile([P, CH * W], f32, tag="tgtc")
                cd = kp.tile([P, CH * W], f32, tag="diffc")
                nf = ns * W
                nc.sync.dma_start(c0[:, :nf], d_out0[:, s0 * W:(s0 + ns) * W])
                nc.sync.dma_start(c1[:, :nf], d_out1[:, s0 * W:(s0 + ns) * W])
                nc.sync.dma_start(ct[:, :nf], d_tgt[:, s0 * W:(s0 + ns) * W])
                nc.vector.tensor_sub(cd[:, :nf], c1[:, :nf], c0[:, :nf])
                # tgt2 = 2*tgt -> bf16 padded slab (Pool TT: t+t; the
                # TensorScalar opcode family is not legal on Pool/v3)
                ctv = ct[:].rearrange("p (s w) -> p s w", w=W)[:, :ns, :]
                nc.gpsimd.tensor_add(tgt2[:, sl, 2:130], ctv, ctv)
                # mc = (diff > 0) + tgt2   in {0,1,2,3}
                nc.vector.scalar_tensor_tensor(
                    mc[:, sl, 2:130],
                    cd[:].rearrange("p (s w) -> p s w", w=W)[:, :ns, :],
                    0.0, tgt2[:, sl, 2:130], op0=Alu.is_gt, op1=Alu.add)
                # probs = sigmoid(diff) on owned slice overlap
                o0 = max(s0, OLO)
                o1 = min(s0 + ns, OHI)
                if o0 < o1:
                    cdv = cd[:].rearrange("p (s w) -> p s w", w=W)
                    nc.scalar.activation(
                        probs[:, o0 - OLO:o1 - OLO, :],
                        cdv[:, o0 - s0:o1 - s0, :], Act.Sigmoid)

            # mc replicated w-pad columns
            nc.gpsimd.tensor_copy(mc[:, :, 1:2], mc[:, :, 2:3])
            nc.gpsimd.tensor_copy(mc[:, :, 130:131], mc[:, :, 129:130])

            # ---- boundary fields ----
            nc.gpsimd.tensor_mul(sq[:, :, 1:131], mc[:, :, 1:131],
                                 mc[:, :, 1:131])
            nc.vector.tensor_add(tb[:, BLO:BHI, 2:130], mc[:, 0:68, 2:130],
                                 mc[:, 2:70, 2:130])
            nc.gpsimd.tensor_add(tbq[:, BLO:BHI, 2:130], sq[:, 0:68, 2:130],
                                 sq[:, 2:70, 2:130])

            # t1 = M_c@mc - 2I@mc[w-1] - 2I@mc[w+1] - 2I@tb   (17 chunks)
            for g in range(17):
                s0 = BLO + g * CH
                sl = slice(s0, s0 + CH)
                pt = ps_t1.tile([P, CH * W], f32, tag="t1ps")
                pt3 = pt[:].rearrange("p (s w) -> p s w", w=W)
                nc.tensor.matmul(pt3[:], mats["m_mc"][:], mc[:, sl, 2:130],
                                 start=True, stop=False)
                nc.tensor.matmul(pt3[:], mats["m_n2i"][:], mc[:, sl, 1:129],
                                 start=False, stop=False)
                nc.tensor.matmul(pt3[:], mats["m_n2i"][:], mc[:, sl, 3:131],
                                 start=False, stop=False)
                nc.tensor.matmul(pt3[:], mats["m_n2i"][:], tb[:, sl, 2:130],
                                 start=False, stop=True)
                nc.scalar.copy(t1[:, g * CH:(g + 1) * CH, :], pt3[:])
                # etmp = mc * t1
                nc.vector.tensor_mul(etmp[:, g * CH:(g + 1) * CH, :],
                                     mc[:, sl, 2:130],
                                     t1[:, g * CH:(g + 1) * CH, :])

            # zero E w-pads (cols 0,1,130,131); D edge slots never read
            nc.gpsimd.memset(e3[:, :, 0:2], 0.0)
            nc.gpsimd.memset(e3[:, :, 130:132], 0.0)

            # E = A1@sq + I@sq[w-1] + I@sq[w+1] + I@tbq + I@etmp
            for g in range(17):
                s0 = BLO + g * CH
                sl = slice(s0, s0 + CH)
                pe_ = ps_e.tile([P, CH * W], f32, tag="eps")
                pe3 = pe_[:].rearrange("p (s w) -> p s w", w=W)
                nc.tensor.matmul(pe3[:], mats["m_a1"][:], sq[:, sl, 2:130],
                                 start=True, stop=False)
                nc.tensor.matmul(pe3[:], mats["m_id"][:], sq[:, sl, 1:129],
                                 start=False, stop=False)
                nc.tensor.matmul(pe3[:], mats["m_id"][:], sq[:, sl, 3:131],
                                 start=False, stop=False)
                nc.tensor.matmul(pe3[:], mats["m_id"][:], tbq[:, sl, 2:130],
                                 start=False, stop=False)
                nc.tensor.matmul(pe3[:], mats["m_id"][:],
                                 etmp[:, g * CH:(g + 1) * CH, :],
                                 start=False, stop=True)
                nc.scalar.copy(e3[:, sl, 2:130], pe3[:])

            # ---- dilation ----
            # reuse dead slabs' slots (same tag -> same memory; Tile adds
            # the WAR deps): mc/sq/tb die with the t1/E matmul phases.
            s3z = slab("s3z", tag_override="mc")
            f3 = slab("f", tag_override="sq")
            c4b = slab("c4b", tag_override="tb")
            nc.vector.tensor_add(s3z[:, 2:68, :], e3[:, 1:67, :],
                                 e3[:, 3:69, :])
            nc.vector.tensor_add(f3[:, 2:68, :], e3[:, 2:68, :],
                                 s3z[:, 2:68, :])
            nc.gpsimd.tensor_add(c4b[:, OLO:OHI, :], e3[:, 1:65, :],
                                 e3[:, 5:69, :])

            r3 = slab("r", cols=W, slots=OWN, tag_override="t1")
            for j in range(16):
                s0 = OLO + j * CH
                sl = slice(s0, s0 + CH)
                pp = ps_p.tile([P, CH * W], f32, tag="pps")
                pp3 = pp[:].rearrange("p (s w) -> p s w", w=W)
                nc.tensor.matmul(pp3[:], mats["m_t5"][:], e3[:, sl, 2:130],
                                 start=True, stop=False)
                nc.tensor.matmul(pp3[:], mats["m_t3"][:], s3z[:, sl, 2:130],
                                 start=False, stop=False)
                nc.tensor.matmul(pp3[:], mats["m_t3"][:], f3[:, sl, 1:129],
                                 start=False, stop=False)
                nc.tensor.matmul(pp3[:], mats["m_t3"][:], f3[:, sl, 3:131],
                                 start=False, stop=False)
                nc.tensor.matmul(pp3[:], mats["m_id"][:], c4b[:, sl, 2:130],
                                 start=False, stop=False)
                nc.tensor.matmul(pp3[:], mats["m_id"][:], e3[:, sl, 0:128],
                                 start=False, stop=False)
                nc.tensor.matmul(pp3[:], mats["m_id"][:], e3[:, sl, 4:132],
                                 start=False, stop=True)
                nc.scalar.copy(r3[:, j * CH:(j + 1) * CH, :], pp3[:])

            # ---- products + fused row sums ----
            pt3_ = slab("pt", cols=W, slots=OWN, tag_override="tbq")
            scr1 = slab("scr1", cols=W, slots=OWN, tag_override="etmp")
            scr2 = slab("scr2", cols=W, slots=OWN, tag_override="mc")
            acc = sp.tile([P, 4], f32, tag="acc")
            tgt_own = tgt2[:, OLO:OHI, 2:130]
            nc.gpsimd.tensor_mul(pt3_[:], probs[:], tgt_own)
            nc.vector.scalar_tensor_tensor(
                scr1[:], r3[:], 0.5, probs[:], op0=Alu.is_gt, op1=Alu.mult,
                accum_out=acc[:, 0:1])
            nc.vector.scalar_tensor_tensor(
                scr1[:], r3[:], 0.5, pt3_[:], op0=Alu.is_gt, op1=Alu.mult,
                accum_out=acc[:, 1:2])
            nc.vector.scalar_tensor_tensor(
                scr2[:], r3[:], 0.5, tgt_own, op0=Alu.is_gt, op1=Alu.mult,
                accum_out=acc[:, 2:3])
            nc.vector.tensor_scalar(
                scr1[:], r3[:], 0.5, None, op0=Alu.is_gt, op1=Alu.add,
                accum_out=acc[:, 3:4])

            nc.sync.dma_start(d_psums[:], acc[:])

    nc.compile()
    return nc


_CACHE = {}
TRACE = False          # test.py sets True for neuron-profile runs
_LAST = {"exec_time_ns": None, "results": None}


def _get_program():
    if "nc" not in _CACHE:
        _CACHE["nc"] = _build_program()
    return _CACHE["nc"]


def last_exec_time_ns():
    return _LAST["exec_time_ns"]


def _core_slabs(output, target, c):
    s, h = c // 2, c % 2
    d0 = 0 if h == 0 else OWN
    sl = slice(d0, d0 + DEXT)
    out_p = np.pad(output[s], ((0, 0), (HALO, HALO), (0, 0), (0, 0)),
                   mode="edge")
    tgt_p = np.pad(target[s, 0], ((HALO, HALO), (0, 0), (0, 0)), mode="edge")

    def tr(a):  # [DEXT,H,W] -> [H, DEXT*W] contiguous
        return np.ascontiguousarray(a.transpose(1, 0, 2)).reshape(P, DEXT * W)

    return {"out0": tr(out_p[0][sl]), "out1": tr(out_p[1][sl]),
            "tgt": tr(tgt_p[sl])}


def kernel(output, target):
    output = np.asarray(output, dtype=np.float32)
    target = np.asarray(target, dtype=np.float32)
    nc = _get_program()

    mats = {n: m.astype(ml_dtypes.bfloat16) for n, m in _const_mats().items()}
    in_maps = []
    for c in range(8):
        m = _core_slabs(output, target, c)
        m.update(mats)
        in_maps.append(m)

    res = run_bass_kernel_spmd(nc, in_maps, list(range(8)), trace=TRACE)
    _LAST["exec_time_ns"] = res.exec_time_ns
    _LAST["results"] = res
    parts = np.zeros((B, 4), np.float64)
    for c in range(8):
        parts[c // 2] += res.results[c]["psums"].astype(np.float64).sum(axis=0)
    s_pm, s_ptm2, s_tm2, s_m = parts.T
    inter = s_ptm2 / 2.0
    card = s_pm + s_tm2 / 2.0
    dice = (2.0 * inter + EPS) / (card + EPS)
    per_sample = np.where(s_m > 0, 1.0 - dice, 0.0)
    return np.float32(per_sample.sum() / B)
